# revision 4
# baseline (speedup 1.0000x reference)
"""Trainium2 Bass kernel for log-softmax multi-head attention (8 NeuronCores).

Reference computation (per batch):
    qkv = x @ w_qkv ; q,k,v per head
    dots = scale * q @ k^T ; attn = log_softmax(dots)
    out = attn @ v  -> merge heads -> out @ w_out + b_out + x

Algebraic identities used:
  1) log_softmax is linear in scores minus a row constant:
       attn = scale*dots - lse,  lse_i = ln sum_j exp(scale*dots_ij)
     so  out_head = scale * q @ (k^T v) - lse (x) colsum(v)
  2) k^T v = Wk^T (x^T x) Wv  (Gram matrix G = x^T x shared by all heads)
  3) colsum(v) = colsum(x) @ Wv
  4) the lse rank-1 correction commutes with the output projection:
       Y -= lnS_h (x) (vsum_h @ Wo_h)   summed over heads
  so the only O(n^2) work is the score matmul + exp/row-sum pass.

Sharding: 8 cores = 2 batches x 4 query-quarters. Every core computes k^T /
G for its full batch and q / lse / output for its own 1024 query rows ->
outputs disjoint, no collectives.

Schedule: the exp+rowsum pass on ScalarE is the hard floor (~265us at
FD=2048: 128 ACTIVATEs over [128,2048] PSUM tiles; accumulator reads overlap
the stream).  FD=2048 double-buffered uses all 8 PSUM banks, so auxiliary
matmul work (kT = Wk^T x^T, the Gram chain, OT precompute, base projections)
"rides" inside the dots buffers: each slot's tile is dead between its
accumulator read and its next fill, so a ride writes dtile[:, 0:512], a DVE
copy drains it, and the dots matmuls overwrite it.  Tile's dependency
tracker orders exp(i-2) -> ride MMs -> ride copy -> dots MMs -> exp(i).
Ln + the rank-1 correction run in a short tail (Exp and Ln live in
different ScalarE table sets; mixing them mid-stream thrashes table loads).
"""

import numpy as np

B, N, D = 2, 4096, 512
H, DH = 8, 64
SCALE = DH**-0.5
NQ = N // 4        # own query rows per core
QT = NQ // 128     # 8 own row tiles
NXT = N // 128     # 32 x row tiles

# Schraudolph fast-exp constants (folding the 1/sqrt(DH) score scale):
#   i32 = trunc(raw * SC1 + SB1); bitcast_f32(i32) ~ exp(SCALE*raw)
# c=486411 zeros the mean relative error (max ~3.9%/elem, ~0.2% on lse)
SC1 = float(np.float32(SCALE * (2.0**23) / np.log(2.0)))
SB1 = float(np.float32(127.0 * 2.0**23 - 486411.0))

_GRAPH_CACHE = {}


def _build_graph():
    import concourse.bass as bass
    import concourse.tile as tile
    from concourse import bacc, mybir
    from concourse.masks import make_identity

    f32 = mybir.dt.float32
    bf16 = mybir.dt.bfloat16
    AF = mybir.ActivationFunctionType

    nc = bacc.Bacc("TRN2", target_bir_lowering=False, debug=False)

    xbf_d = nc.dram_tensor("x_bf", [N, D], bf16, kind="ExternalInput").ap()
    xq_d = nc.dram_tensor("xq", [NQ, D], f32, kind="ExternalInput").ap()
    wqkv_d = nc.dram_tensor("w_qkv_bf", [D, 3 * D], bf16, kind="ExternalInput").ap()
    wout_d = nc.dram_tensor("w_out_bf", [D, D], bf16, kind="ExternalInput").ap()
    bout_d = nc.dram_tensor("b_out", [D], f32, kind="ExternalInput").ap()
    out_d = nc.dram_tensor("out", [NQ, D], f32, kind="ExternalOutput").ap()

    with tile.TileContext(nc) as tc:
        with (
            tc.tile_pool(name="const", bufs=1) as const,
            tc.tile_pool(name="bigsb", bufs=1) as bigsb,
            tc.tile_pool(name="dout", bufs=2) as dout,
        ):
            # ------- constants + DMAs (issue order = priority) --------------
            ident_bf = const.tile([128, 128], bf16, tag="ident_bf")
            make_identity(nc, ident_bf[:])
            b_bc = const.tile([128, D], f32, tag="b_bc")
            nc.sync.dma_start(
                out=b_bc[:],
                in_=bass.AP(
                    tensor=bout_d.tensor,
                    offset=bout_d.offset,
                    ap=[[0, 128]] + [list(p) for p in bout_d.ap],
                ),
            )
            wq = []
            for j in range(4):
                w_t = const.tile([128, 3 * D], bf16, tag=f"wq{j}")
                nc.sync.dma_start(out=w_t[:], in_=wqkv_d[j * 128 : (j + 1) * 128, :])
                wq.append(w_t)

            # coarse 1024-row transpose pieces: the serialized HWDGE issue
            # cost is ~flat per instruction, so 16 DMAs beat 32
            xT = [bigsb.tile([128, N], bf16, name=f"xT{j}", tag=f"xT{j}") for j in range(4)]
            for r in range(4):
                for j in range(4):
                    nc.sync.dma_start(
                        out=xT[j][:, r * 1024 : (r + 1) * 1024],
                        in_=xbf_d[r * 1024 : (r + 1) * 1024, j * 128 : (j + 1) * 128],
                        transpose=True,
                    )
            wo = []
            for j in range(4):
                w_t = const.tile([128, D], bf16, tag=f"wo{j}")
                nc.sync.dma_start(out=w_t[:], in_=wout_d[j * 128 : (j + 1) * 128, :])
                wo.append(w_t)
            # residual rows (f32) -> become x + b via DVE adds under stream
            xb = []
            for t in range(QT):
                xb_t = dout.tile([128, D], f32, tag=f"xb{t}", bufs=1)
                nc.sync.dma_start(out=xb_t[:], in_=xq_d[t * 128 : (t + 1) * 128, :])
                xb.append(xb_t)
            # x row tiles (Gram matrix; consumed only after the stream, so
            # these DMAs are issued last and land during the exp stream)
            xrow = []
            for t in range(NXT):
                xr_t = bigsb.tile([128, D], bf16, tag=f"xrow{t}")
                nc.sync.dma_start(out=xr_t[:], in_=xbf_d[t * 128 : (t + 1) * 128, :])
                xrow.append(xr_t)

            # ------- big SBUF operands --------------------------------------
            qT = [bigsb.tile([128, NQ], bf16, name=f"qT{c}", tag=f"qT{c}") for c in range(4)]
            kT = [bigsb.tile([128, N], bf16, name=f"kT{c}", tag=f"kT{c}") for c in range(4)]
            G_sb = [bigsb.tile([128, D], f32, name=f"G{j}", tag=f"G{j}") for j in range(4)]
            G_bf = [bigsb.tile([128, D], bf16, name=f"Gb{j}", tag=f"Gb{j}") for j in range(4)]
            GWk = [bigsb.tile([128, D], bf16, name=f"GWk{j}", tag=f"GWk{j}") for j in range(4)]
            KVW = [bigsb.tile([128, D], bf16, name=f"KVW{c}", tag=f"KVW{c}") for c in range(4)]
            kv_p = const.tile([128, D], bf16, tag="kv_p")
            nc.vector.memset(kv_p[:], 0.0)
            csx4 = [const.tile([128, 4], f32, name=f"csx4_{j}", tag=f"csx4_{j}") for j in range(4)]
            csx_bf = [const.tile([128, 1], bf16, name=f"csxb{j}", tag=f"csxb{j}") for j in range(4)]
            vsT = [const.tile([128, 1], bf16, name=f"vsT{j}", tag=f"vsT{j}") for j in range(4)]
            VSmat = [const.tile([128, 8], bf16, name=f"VSm{j}", tag=f"VSm{j}") for j in range(4)]
            for j in range(4):
                nc.vector.memset(VSmat[j][:], 0.0)
            W8_sb = const.tile([8, D], bf16, tag="W8")
            lse_acc = const.tile([128, 128], f32, tag="lse_acc")
            lse_sum = const.tile([128, 64], f32, tag="lse_sum")
            lse_ln = const.tile([128, 64], bf16, tag="lse_ln")
            # DVE bit-trick exp scratch: i32 = trunc(raw*SC1 + SB1);
            # bitcast-f32(i32) ~= exp(SCALE*raw) (Schraudolph, c=486411)
            scr_i32 = const.tile([128, 2048], mybir.dt.int32, tag="scr_i32")
            scr_out = const.tile([128, 2048], bf16, tag="scr_out")
            lnST = const.tile([8, NQ], bf16, tag="lnST")
            dummy = const.tile([128, 1], f32, tag="dummy")
            nc.vector.memset(dummy[:], 0.0)

            # preload the Exp table set before the stream
            nc.scalar.activation(out=dummy[:], in_=dummy[:], func=AF.Exp)
            if True:

                # ---- ride bodies: scr is a [128,512] f32 psum view ---------
                def qT_half(scr, c, nn):
                    for j in range(4):
                        nc.tensor.matmul(
                            scr,
                            lhsT=wq[j][:, c * 128 : (c + 1) * 128],
                            rhs=xT[j][:, nn * 512 : (nn + 1) * 512],
                            start=(j == 0),
                            stop=(j == 3),
                        )
                    nc.vector.tensor_copy(qT[c][:, nn * 512 : (nn + 1) * 512], scr)

                def kT_chunk(scr, c, ch):
                    for j in range(4):
                        nc.tensor.matmul(
                            scr,
                            lhsT=wq[j][:, 512 + c * 128 : 512 + (c + 1) * 128],
                            rhs=xT[j][:, ch * 512 : (ch + 1) * 512],
                            start=(j == 0),
                            stop=(j == 3),
                        )
                    nc.vector.tensor_copy(kT[c][:, ch * 512 : (ch + 1) * 512], scr)

                def g_chunk(scr, jm, t0):
                    for t in range(t0, t0 + 4):
                        nc.tensor.matmul(
                            scr,
                            lhsT=xrow[t][:, jm * 128 : (jm + 1) * 128],
                            rhs=xrow[t][:],
                            start=(t == t0),
                            stop=(t == t0 + 3),
                        )
                    if t0 == 0:
                        nc.vector.tensor_copy(G_sb[jm][:], scr)
                    else:
                        nc.vector.tensor_add(G_sb[jm][:], G_sb[jm][:], scr)

                def g_fin(jm):
                    nc.vector.tensor_copy(G_bf[jm][:], G_sb[jm][:])

                def gwk_jm(scr, jm):
                    # GWk[jm] = (G @ Wk) rows jm*128:(jm+1)*128
                    for j in range(4):
                        nc.tensor.matmul(
                            scr,
                            lhsT=G_bf[j][:, jm * 128 : (jm + 1) * 128],
                            rhs=wq[j][:, 512:1024],
                            start=(j == 0),
                            stop=(j == 3),
                        )
                    nc.vector.tensor_copy(GWk[jm][:], scr)

                def kvt_head(scr, h):
                    # kv^T_h = Wv_h^T (G Wk)_h ; scaled into kv_p rows r0
                    r0 = (h % 2) * 64
                    for j in range(4):
                        nc.tensor.matmul(
                            scr[0:64, 0:64],
                            lhsT=wq[j][:, 1024 + h * 64 : 1024 + (h + 1) * 64],
                            rhs=GWk[j][:, h * 64 : (h + 1) * 64],
                            start=(j == 0),
                            stop=(j == 3),
                        )
                    nc.vector.tensor_scalar_mul(
                        kv_p[r0 : r0 + 64, h * 64 : (h + 1) * 64], scr[0:64, 0:64], SCALE
                    )

                def kvw_c(scr, c):
                    # KVW_c[b-rows, :] = scale * kv_h @ Wo_h for both heads of c
                    for hp in range(2):
                        h, r0 = 2 * c + hp, hp * 64
                        nc.tensor.matmul(
                            scr[r0 : r0 + 64, :],
                            lhsT=kv_p[:, h * 64 : (h + 1) * 64],
                            rhs=wo[c][:],
                            start=True,
                            stop=True,
                        )
                    nc.vector.tensor_copy(KVW[c][:], scr)

                def csx_piece(scr, j, p):
                    nc.vector.tensor_reduce(
                        csx4[j][:, p : p + 1],
                        xT[j][:, p * 1024 : (p + 1) * 1024],
                        axis=mybir.AxisListType.X,
                        op=mybir.AluOpType.add,
                    )

                def csx_fin(scr, j):
                    nc.vector.tensor_reduce(
                        csx4[j][:, 0:1], csx4[j][:],
                        axis=mybir.AxisListType.X, op=mybir.AluOpType.add,
                    )
                    nc.vector.tensor_copy(csx_bf[j][:], csx4[j][:, 0:1])

                def vsum_jm(scr, jm):
                    # vsT[jm] = -(Wv^T colsum(x)) block jm (minus sign -> W8)
                    for j in range(4):
                        nc.tensor.matmul(
                            scr[:, 0:1],
                            lhsT=wq[j][:, 1024 + jm * 128 : 1024 + (jm + 1) * 128],
                            rhs=csx_bf[j][:],
                            start=(j == 0),
                            stop=(j == 3),
                        )
                    nc.vector.tensor_scalar_mul(vsT[jm][:], scr[:, 0:1], -1.0)
                    nc.vector.tensor_copy(
                        VSmat[jm][0:64, 2 * jm : 2 * jm + 1], vsT[jm][0:64, :]
                    )
                    nc.vector.tensor_copy(
                        VSmat[jm][64:128, 2 * jm + 1 : 2 * jm + 2], vsT[jm][64:128, :]
                    )

                def w8_mm(scr):
                    for j in range(4):
                        nc.tensor.matmul(
                            scr[0:8, :],
                            lhsT=VSmat[j][:],
                            rhs=wo[j][:],
                            start=(j == 0),
                            stop=(j == 3),
                        )
                    nc.vector.tensor_copy(W8_sb[:], scr[0:8, :])

                def xb_add(scr, t):
                    nc.vector.tensor_add(xb[t][:], xb[t][:], b_bc[:])

                # ---- ride schedule (FIFO; one MM ride per stream slot,
                # DVE-only rides drain from their own queue) -----------------
                head_rides = []   # strict xT r-wave arrival order so the
                tail_rides = []   # head never waits on a late transpose wave
                dve_rides = []
                for ch in range(8):
                    if ch < 2:
                        for c in range(1, 4):
                            head_rides.append(
                                lambda s, c=c, nn=ch: qT_half(s, c, nn)
                            )
                    for c in range(1, 4):
                        head_rides.append(lambda s, c=c, ch=ch: kT_chunk(s, c, ch))
                    if ch >= 4:
                        head_rides.append(lambda s, ch=ch: kT_chunk(s, 0, ch))
                for jm in range(4):                       # Gram matrix
                    for t0 in range(0, NXT, 4):
                        tail_rides.append(lambda s, jm=jm, t0=t0: g_chunk(s, jm, t0))
                    tail_rides.append(lambda s, jm=jm: g_fin(jm))
                for j in range(4):
                    for p in range(4):
                        dve_rides.append(lambda j=j, p=p: csx_piece(None, j, p))
                for j in range(4):
                    dve_rides.append(lambda j=j: csx_fin(None, j))
                for t in range(QT):
                    dve_rides.append(lambda t=t: xb_add(None, t))
                for jm in range(4):                       # GWk (bf16)
                    tail_rides.append(lambda s, jm=jm: gwk_jm(s, jm))
                for h in range(H):
                    tail_rides.append(lambda s, h=h: kvt_head(s, h))
                for jm in range(4):
                    tail_rides.append(lambda s, jm=jm: vsum_jm(s, jm))
                tail_rides.append(lambda s: w8_mm(s))
                for c in range(4):
                    tail_rides.append(lambda s, c=c: kvw_c(s, c))

            # ---- head: all aux units on a deep-buffered scratch pool ------
            with tc.tile_pool(name="head_ps", bufs=1, space="PSUM") as hps:
                def head_tile():
                    return hps.tile([128, 512], f32, name="hsc", tag="hsc", bufs=8)

                for nn in range(2):
                    qT_half(head_tile()[:], 0, nn)
                for ch in range(4):
                    kT_chunk(head_tile()[:], 0, ch)
                while head_rides:
                    head_rides.pop(0)(head_tile()[:])

            # ---- the exp stream: 128 slots of FD=2048 ----------------------
            with tc.tile_pool(name="dots_ps", bufs=1, space="PSUM") as dps:
                def new_tile():
                    return dps.tile([128, 2048], f32, name="dots", tag="dots", bufs=2)
                for h in range(H):
                    c, r0 = h // 2, (h % 2) * 64
                    for t in range(QT):
                        lhsT = qT[c][r0 : r0 + 64, t * 128 : (t + 1) * 128]
                        for half in range(2):
                            dtile = new_tile()
                            for cc in range(4):
                                nc.tensor.matmul(
                                    dtile[:, cc * 512 : (cc + 1) * 512],
                                    lhsT=lhsT,
                                    rhs=kT[c][
                                        r0 : r0 + 64,
                                        (half * 4 + cc) * 512 : (half * 4 + cc + 1) * 512,
                                    ],
                                    start=True,
                                    stop=True,
                                )
                            col = (h * 8 + t) * 2 + half
                            idx = col
                            # balance exp work: ACT exact-exp tile costs
                            # ~2.16us, DVE 2-pass bit-trick ~3.4us -> DVE
                            # takes 5/13 of tiles (Bresenham interleave)
                            if (idx * 5) // 13 != ((idx + 1) * 5) // 13:
                                nc.vector.tensor_scalar(
                                    out=scr_i32[:],
                                    in0=dtile[:],
                                    scalar1=SC1,
                                    scalar2=SB1,
                                    op0=mybir.AluOpType.mult,
                                    op1=mybir.AluOpType.add,
                                )
                                nc.vector.tensor_scalar(
                                    out=scr_out[:],
                                    in0=scr_i32[:].bitcast(f32),
                                    scalar1=1.0,
                                    scalar2=None,
                                    op0=mybir.AluOpType.mult,
                                    op1=mybir.AluOpType.add,
                                    accum_out=lse_acc[:, col : col + 1],
                                )
                            else:
                                nc.scalar.activation(
                                    out=dtile[:],
                                    in_=dtile[:],
                                    func=AF.Exp,
                                    scale=SCALE,
                                    accum_out=lse_acc[:, col : col + 1],
                                )

            # ---- tail: Gram-chain aux (deep-buffered) overlapped with the
            # lse -> Ln scalar work, then rank-1 + residual ------------------
            with tc.tile_pool(name="tail_ps", bufs=1, space="PSUM") as tps:
                def tail_tile():
                    return tps.tile([128, 512], f32, name="tsc", tag="tsc", bufs=8)

                # scalar/DVE lse work first: overlaps the PE aux below
                la = lse_acc[:].rearrange("q (p two) -> q p two", two=2)
                nc.vector.tensor_add(lse_sum[:], la[:, :, 0], la[:, :, 1])
                nc.scalar.activation(out=lse_ln[:], in_=lse_sum[:], func=AF.Ln)
                # lse_ln cols are h*8+t; gather per-t slices into t-major
                lse_tm = const.tile([128, 64], bf16, tag="lse_tm")
                nc.vector.tensor_copy(
                    lse_tm[:],
                    lse_ln[:].rearrange("q (h t) -> q t h", t=QT),
                )
                nu = 0
                while tail_rides:
                    tail_rides.pop(0)(tail_tile()[:])
                    nu += 1
                    if dve_rides and nu % 2 == 0:
                        dve_rides.pop(0)()
                while dve_rides:
                    dve_rides.pop(0)()
                for t in range(QT):
                    ps = tail_tile()
                    ps_bf = ps[0:8, 0:64].bitcast(bf16)
                    nc.tensor.transpose(ps_bf, lse_tm[:, t * 8 : (t + 1) * 8], ident_bf[:])
                    nc.vector.tensor_copy(lnST[:, t * 128 : (t + 1) * 128], ps_bf)
                    yps = tail_tile()
                    for c in range(4):
                        nc.tensor.matmul(
                            yps[:],
                            lhsT=qT[c][:, t * 128 : (t + 1) * 128],
                            rhs=KVW[c][:],
                            start=(c == 0),
                            stop=False,
                        )
                    nc.tensor.matmul(
                        yps[:],
                        lhsT=lnST[:, t * 128 : (t + 1) * 128],
                        rhs=W8_sb[:],
                        start=False,
                        stop=True,
                    )
                    ysb = dout.tile([128, D], f32, name="ysb", tag="ysb")
                    nc.vector.tensor_add(ysb[:], yps[:], xb[t][:])
                    nc.sync.dma_start(out=out_d[t * 128 : (t + 1) * 128, :], in_=ysb[:])

    nc.compile()
    return nc


def get_graph():
    if "nc" not in _GRAPH_CACHE:
        _GRAPH_CACHE["nc"] = _build_graph()
    return _GRAPH_CACHE["nc"]


def make_in_maps(x, w_qkv, w_out, b_out):
    import ml_dtypes

    x = np.ascontiguousarray(x, dtype=np.float32)
    w_qkv = np.ascontiguousarray(w_qkv, dtype=np.float32)
    w_out = np.ascontiguousarray(w_out, dtype=np.float32)
    b_out = np.ascontiguousarray(b_out, dtype=np.float32)
    x_bf = x.astype(ml_dtypes.bfloat16)
    w_qkv_bf = w_qkv.astype(ml_dtypes.bfloat16)
    w_out_bf = w_out.astype(ml_dtypes.bfloat16)
    in_maps = []
    for i in range(8):
        b, q = divmod(i, 4)
        in_maps.append(
            {
                # keys are permutation-invariant for lse/kv/G; roll so this
                # core's own query rows sit at rows 0:NQ
                "x_bf": np.ascontiguousarray(np.roll(x_bf[b], -q * NQ, axis=0)),
                "xq": np.ascontiguousarray(x[b, q * NQ : (q + 1) * NQ]),
                "w_qkv_bf": w_qkv_bf,
                "w_out_bf": w_out_bf,
                "b_out": b_out,
            }
        )
    return in_maps


def kernel(x, w_qkv, w_out, b_out):
    from concourse.bass_utils import run_bass_kernel_spmd

    nc = get_graph()
    in_maps = make_in_maps(x, w_qkv, w_out, b_out)
    res = run_bass_kernel_spmd(nc, in_maps, core_ids=list(range(8)))
    out = np.empty((B, N, D), np.float32)
    for i in range(8):
        b, q = divmod(i, 4)
        out[b, q * NQ : (q + 1) * NQ] = res.results[i]["out"]
    return out



# revision 9
# speedup vs baseline: 1.0070x; 1.0070x over previous
"""Trainium2 Bass kernel for log-softmax multi-head attention (8 NeuronCores).

Reference computation (per batch):
    qkv = x @ w_qkv ; q,k,v per head
    dots = scale * q @ k^T ; attn = log_softmax(dots)
    out = attn @ v  -> merge heads -> out @ w_out + b_out + x

Algebraic identities used:
  1) log_softmax is linear in scores minus a row constant:
       attn = scale*dots - lse,  lse_i = ln sum_j exp(scale*dots_ij)
     so  out_head = scale * q @ (k^T v) - lse (x) colsum(v)
  2) k^T v = Wk^T (x^T x) Wv  (Gram matrix G = x^T x shared by all heads)
  3) colsum(v) = colsum(x) @ Wv
  4) the lse rank-1 correction commutes with the output projection:
       Y -= lnS_h (x) (vsum_h @ Wo_h)   summed over heads
  so the only O(n^2) work is the score matmul + exp/row-sum pass.

Sharding: 8 cores = 2 batches x 4 query-quarters. Every core computes k^T /
G for its full batch and q / lse / output for its own 1024 query rows ->
outputs disjoint, no collectives.

Schedule: the exp+rowsum pass on ScalarE is the hard floor (~265us at
FD=2048: 128 ACTIVATEs over [128,2048] PSUM tiles; accumulator reads overlap
the stream).  FD=2048 double-buffered uses all 8 PSUM banks, so auxiliary
matmul work (kT = Wk^T x^T, the Gram chain, OT precompute, base projections)
"rides" inside the dots buffers: each slot's tile is dead between its
accumulator read and its next fill, so a ride writes dtile[:, 0:512], a DVE
copy drains it, and the dots matmuls overwrite it.  Tile's dependency
tracker orders exp(i-2) -> ride MMs -> ride copy -> dots MMs -> exp(i).
Ln + the rank-1 correction run in a short tail (Exp and Ln live in
different ScalarE table sets; mixing them mid-stream thrashes table loads).
"""

import numpy as np

B, N, D = 2, 4096, 512
H, DH = 8, 64
SCALE = DH**-0.5
NQ = N // 4        # own query rows per core
QT = NQ // 128     # 8 own row tiles
NXT = N // 128     # 32 x row tiles

# Schraudolph fast-exp in bf16 (folding the 1/sqrt(DH) score scale):
#   i16 = trunc(raw * SC16 + SB16); bitcast_bf16(i16) ~ exp(SCALE*raw)
# c16=6.9 zeros the mean relative error (max ~4%/elem, ~0.2% on lse);
# 16-bit output keeps pass-2 operands packed 2-byte for DVE 2x mode
SC16 = float(np.float32(SCALE * (2.0**7) / np.log(2.0)))
SB16 = float(np.float32(127.0 * 2.0**7 - 6.9))

_GRAPH_CACHE = {}


def _build_graph():
    import concourse.bass as bass
    import concourse.tile as tile
    from concourse import bacc, mybir
    from concourse.masks import make_identity

    f32 = mybir.dt.float32
    bf16 = mybir.dt.bfloat16
    AF = mybir.ActivationFunctionType

    nc = bacc.Bacc("TRN2", target_bir_lowering=False, debug=False)

    xbf_d = nc.dram_tensor("x_bf", [N, D], bf16, kind="ExternalInput").ap()
    xq_d = nc.dram_tensor("xq", [NQ, D], f32, kind="ExternalInput").ap()
    wqkv_d = nc.dram_tensor("w_qkv_bf", [D, 3 * D], bf16, kind="ExternalInput").ap()
    wout_d = nc.dram_tensor("w_out_bf", [D, D], bf16, kind="ExternalInput").ap()
    bout_d = nc.dram_tensor("b_out", [D], f32, kind="ExternalInput").ap()
    out_d = nc.dram_tensor("out", [NQ, D], f32, kind="ExternalOutput").ap()

    with tile.TileContext(nc) as tc:
        with (
            tc.tile_pool(name="const", bufs=1) as const,
            tc.tile_pool(name="bigsb", bufs=1) as bigsb,
            tc.tile_pool(name="dout", bufs=2) as dout,
        ):
            # ------- constants + DMAs (issue order = priority) --------------
            ident_bf = const.tile([128, 128], bf16, tag="ident_bf")
            make_identity(nc, ident_bf[:])
            b_bc = const.tile([128, D], f32, tag="b_bc")
            nc.sync.dma_start(
                out=b_bc[:],
                in_=bass.AP(
                    tensor=bout_d.tensor,
                    offset=bout_d.offset,
                    ap=[[0, 128]] + [list(p) for p in bout_d.ap],
                ),
            )
            wq = []
            for j in range(4):
                w_t = const.tile([128, 3 * D], bf16, tag=f"wq{j}")
                nc.sync.dma_start(out=w_t[:], in_=wqkv_d[j * 128 : (j + 1) * 128, :])
                wq.append(w_t)

            # coarse 1024-row transpose pieces: the serialized HWDGE issue
            # cost is ~flat per instruction, so 16 DMAs beat 32
            xT = [bigsb.tile([128, N], bf16, name=f"xT{j}", tag=f"xT{j}") for j in range(4)]
            for r in range(4):
                for j in range(4):
                    nc.sync.dma_start(
                        out=xT[j][:, r * 1024 : (r + 1) * 1024],
                        in_=xbf_d[r * 1024 : (r + 1) * 1024, j * 128 : (j + 1) * 128],
                        transpose=True,
                    )
            wo = []
            for j in range(4):
                w_t = const.tile([128, D], bf16, tag=f"wo{j}")
                nc.sync.dma_start(out=w_t[:], in_=wout_d[j * 128 : (j + 1) * 128, :])
                wo.append(w_t)
            # residual rows (f32) -> become x + b via DVE adds under stream
            xb = []
            for t in range(QT):
                xb_t = dout.tile([128, D], f32, tag=f"xb{t}", bufs=1)
                nc.sync.dma_start(out=xb_t[:], in_=xq_d[t * 128 : (t + 1) * 128, :])
                xb.append(xb_t)
            # x row tiles (Gram matrix; consumed only after the stream, so
            # these DMAs are issued last and land during the exp stream)
            xrow = []
            for t in range(NXT):
                xr_t = bigsb.tile([128, D], bf16, tag=f"xrow{t}")
                nc.sync.dma_start(out=xr_t[:], in_=xbf_d[t * 128 : (t + 1) * 128, :])
                xrow.append(xr_t)

            # ------- big SBUF operands --------------------------------------
            qT = [bigsb.tile([128, NQ], bf16, name=f"qT{c}", tag=f"qT{c}") for c in range(4)]
            kT = [bigsb.tile([128, N], bf16, name=f"kT{c}", tag=f"kT{c}") for c in range(4)]
            G_sb = [bigsb.tile([128, D], f32, name=f"G{j}", tag=f"G{j}") for j in range(4)]
            G_bf = [bigsb.tile([128, D], bf16, name=f"Gb{j}", tag=f"Gb{j}") for j in range(4)]
            GWk = [bigsb.tile([128, D], bf16, name=f"GWk{j}", tag=f"GWk{j}") for j in range(4)]
            KVW = [bigsb.tile([128, D], bf16, name=f"KVW{c}", tag=f"KVW{c}") for c in range(4)]
            kv_p = const.tile([128, D], bf16, tag="kv_p")
            nc.vector.memset(kv_p[:], 0.0)
            csx4 = [const.tile([128, 4], f32, name=f"csx4_{j}", tag=f"csx4_{j}") for j in range(4)]
            csx_bf = [const.tile([128, 1], bf16, name=f"csxb{j}", tag=f"csxb{j}") for j in range(4)]
            vsT = [const.tile([128, 1], bf16, name=f"vsT{j}", tag=f"vsT{j}") for j in range(4)]
            VSmat = [const.tile([128, 8], bf16, name=f"VSm{j}", tag=f"VSm{j}") for j in range(4)]
            for j in range(4):
                nc.vector.memset(VSmat[j][:], 0.0)
            W8_sb = const.tile([8, D], bf16, tag="W8")
            lse_acc = const.tile([128, 128], f32, tag="lse_acc")
            lse_sum = const.tile([128, 64], f32, tag="lse_sum")
            lse_ln = const.tile([128, 64], bf16, tag="lse_ln")
            # DVE bit-trick exp scratch (bf16 Schraudolph)
            scr_i16 = const.tile([128, 2048], mybir.dt.int16, tag="scr_i16")
            scr_out = const.tile([128, 2048], bf16, tag="scr_out")
            lnST = const.tile([8, NQ], bf16, tag="lnST")
            dummy = const.tile([128, 1], f32, tag="dummy")
            nc.vector.memset(dummy[:], 0.0)

            # preload the Exp table set before the stream
            nc.scalar.activation(out=dummy[:], in_=dummy[:], func=AF.Exp)
            if True:

                # ---- ride bodies: scr is a [128,512] f32 psum view ---------
                def qT_half(scr, c, nn):
                    for j in range(4):
                        nc.tensor.matmul(
                            scr,
                            lhsT=wq[j][:, c * 128 : (c + 1) * 128],
                            rhs=xT[j][:, nn * 512 : (nn + 1) * 512],
                            start=(j == 0),
                            stop=(j == 3),
                        )
                    nc.vector.tensor_copy(qT[c][:, nn * 512 : (nn + 1) * 512], scr)

                def kT_chunk(scr, c, ch):
                    for j in range(4):
                        nc.tensor.matmul(
                            scr,
                            lhsT=wq[j][:, 512 + c * 128 : 512 + (c + 1) * 128],
                            rhs=xT[j][:, ch * 512 : (ch + 1) * 512],
                            start=(j == 0),
                            stop=(j == 3),
                        )
                    nc.vector.tensor_copy(kT[c][:, ch * 512 : (ch + 1) * 512], scr)

                def g_chunk(scr, jm, t0):
                    for t in range(t0, t0 + 4):
                        nc.tensor.matmul(
                            scr,
                            lhsT=xrow[t][:, jm * 128 : (jm + 1) * 128],
                            rhs=xrow[t][:],
                            start=(t == t0),
                            stop=(t == t0 + 3),
                        )
                    if t0 == 0:
                        nc.vector.tensor_copy(G_sb[jm][:], scr)
                    else:
                        nc.vector.tensor_add(G_sb[jm][:], G_sb[jm][:], scr)

                def g_fin(jm):
                    nc.vector.tensor_copy(G_bf[jm][:], G_sb[jm][:])

                def gwk_jm(scr, jm):
                    # GWk[jm] = (G @ Wk) rows jm*128:(jm+1)*128
                    for j in range(4):
                        nc.tensor.matmul(
                            scr,
                            lhsT=G_bf[j][:, jm * 128 : (jm + 1) * 128],
                            rhs=wq[j][:, 512:1024],
                            start=(j == 0),
                            stop=(j == 3),
                        )
                    nc.vector.tensor_copy(GWk[jm][:], scr)

                def kvt_head(scr, h):
                    # kv^T_h = Wv_h^T (G Wk)_h ; scaled into kv_p rows r0
                    r0 = (h % 2) * 64
                    for j in range(4):
                        nc.tensor.matmul(
                            scr[0:64, 0:64],
                            lhsT=wq[j][:, 1024 + h * 64 : 1024 + (h + 1) * 64],
                            rhs=GWk[j][:, h * 64 : (h + 1) * 64],
                            start=(j == 0),
                            stop=(j == 3),
                        )
                    nc.vector.tensor_scalar_mul(
                        kv_p[r0 : r0 + 64, h * 64 : (h + 1) * 64], scr[0:64, 0:64], SCALE
                    )

                def kvw_c(scr, c):
                    # KVW_c[b-rows, :] = scale * kv_h @ Wo_h for both heads of c
                    for hp in range(2):
                        h, r0 = 2 * c + hp, hp * 64
                        nc.tensor.matmul(
                            scr[r0 : r0 + 64, :],
                            lhsT=kv_p[:, h * 64 : (h + 1) * 64],
                            rhs=wo[c][:],
                            start=True,
                            stop=True,
                        )
                    nc.vector.tensor_copy(KVW[c][:], scr)

                def csx_piece(scr, j, p):
                    nc.vector.tensor_reduce(
                        csx4[j][:, p : p + 1],
                        xT[j][:, p * 1024 : (p + 1) * 1024],
                        axis=mybir.AxisListType.X,
                        op=mybir.AluOpType.add,
                    )

                def csx_fin(scr, j):
                    nc.vector.tensor_reduce(
                        csx4[j][:, 0:1], csx4[j][:],
                        axis=mybir.AxisListType.X, op=mybir.AluOpType.add,
                    )
                    nc.vector.tensor_copy(csx_bf[j][:], csx4[j][:, 0:1])

                def vsum_jm(scr, jm):
                    # vsT[jm] = -(Wv^T colsum(x)) block jm (minus sign -> W8)
                    for j in range(4):
                        nc.tensor.matmul(
                            scr[:, 0:1],
                            lhsT=wq[j][:, 1024 + jm * 128 : 1024 + (jm + 1) * 128],
                            rhs=csx_bf[j][:],
                            start=(j == 0),
                            stop=(j == 3),
                        )
                    nc.vector.tensor_scalar_mul(vsT[jm][:], scr[:, 0:1], -1.0)
                    nc.vector.tensor_copy(
                        VSmat[jm][0:64, 2 * jm : 2 * jm + 1], vsT[jm][0:64, :]
                    )
                    nc.vector.tensor_copy(
                        VSmat[jm][64:128, 2 * jm + 1 : 2 * jm + 2], vsT[jm][64:128, :]
                    )

                def w8_mm(scr):
                    for j in range(4):
                        nc.tensor.matmul(
                            scr[0:8, :],
                            lhsT=VSmat[j][:],
                            rhs=wo[j][:],
                            start=(j == 0),
                            stop=(j == 3),
                        )
                    nc.vector.tensor_copy(W8_sb[:], scr[0:8, :])

                def xb_add(scr, t):
                    nc.vector.tensor_add(xb[t][:], xb[t][:], b_bc[:])

                # ---- ride schedule (FIFO; one MM ride per stream slot,
                # DVE-only rides drain from their own queue) -----------------
                head_rides = []   # strict xT r-wave arrival order so the
                tail_rides = []   # head never waits on a late transpose wave
                dve_rides = []
                for ch in range(8):
                    if ch < 2:
                        for c in range(1, 4):
                            head_rides.append(
                                lambda s, c=c, nn=ch: qT_half(s, c, nn)
                            )
                    for c in range(1, 4):
                        head_rides.append(lambda s, c=c, ch=ch: kT_chunk(s, c, ch))
                    if ch >= 4:
                        head_rides.append(lambda s, ch=ch: kT_chunk(s, 0, ch))
                for jm in range(4):                       # Gram matrix
                    for t0 in range(0, NXT, 4):
                        tail_rides.append(lambda s, jm=jm, t0=t0: g_chunk(s, jm, t0))
                    tail_rides.append(lambda s, jm=jm: g_fin(jm))
                for j in range(4):
                    for p in range(4):
                        dve_rides.append(lambda j=j, p=p: csx_piece(None, j, p))
                for j in range(4):
                    dve_rides.append(lambda j=j: csx_fin(None, j))
                for t in range(QT):
                    dve_rides.append(lambda t=t: xb_add(None, t))
                for jm in range(4):                       # GWk (bf16)
                    tail_rides.append(lambda s, jm=jm: gwk_jm(s, jm))
                for h in range(H):
                    tail_rides.append(lambda s, h=h: kvt_head(s, h))
                for jm in range(4):
                    tail_rides.append(lambda s, jm=jm: vsum_jm(s, jm))
                tail_rides.append(lambda s: w8_mm(s))
                for c in range(4):
                    tail_rides.append(lambda s, c=c: kvw_c(s, c))

            # ---- head: all aux units on a deep-buffered scratch pool ------
            with tc.tile_pool(name="head_ps", bufs=1, space="PSUM") as hps:
                def head_tile():
                    return hps.tile([128, 512], f32, name="hsc", tag="hsc", bufs=8)

                for nn in range(2):
                    qT_half(head_tile()[:], 0, nn)
                for ch in range(4):
                    kT_chunk(head_tile()[:], 0, ch)
                while head_rides:
                    head_rides.pop(0)(head_tile()[:])

            # ---- the exp stream: 128 slots of FD=2048 ----------------------
            with tc.tile_pool(name="dots_ps", bufs=1, space="PSUM") as dps:
                def new_tile():
                    return dps.tile([128, 2048], f32, name="dots", tag="dots", bufs=2)
                for h in range(H):
                    c, r0 = h // 2, (h % 2) * 64
                    for t in range(QT):
                        lhsT = qT[c][r0 : r0 + 64, t * 128 : (t + 1) * 128]
                        for half in range(2):
                            dtile = new_tile()
                            for cc in range(4):
                                nc.tensor.matmul(
                                    dtile[:, cc * 512 : (cc + 1) * 512],
                                    lhsT=lhsT,
                                    rhs=kT[c][
                                        r0 : r0 + 64,
                                        (half * 4 + cc) * 512 : (half * 4 + cc + 1) * 512,
                                    ],
                                    start=True,
                                    stop=True,
                                )
                            col = (h * 8 + t) * 2 + half
                            idx = col
                            # balance exp work: ACT exact-exp ~2.18us/tile,
                            # DVE 2-pass bf16 bit-trick -> DVE takes 46/128
                            if (idx * 46) // 128 != ((idx + 1) * 46) // 128:
                                nc.vector.tensor_scalar(
                                    out=scr_i16[:],
                                    in0=dtile[:],
                                    scalar1=SC16,
                                    scalar2=SB16,
                                    op0=mybir.AluOpType.mult,
                                    op1=mybir.AluOpType.add,
                                )
                                nc.vector.tensor_scalar(
                                    out=scr_out[:],
                                    in0=scr_i16[:].bitcast(bf16),
                                    scalar1=1.0,
                                    scalar2=None,
                                    op0=mybir.AluOpType.mult,
                                    op1=mybir.AluOpType.add,
                                    accum_out=lse_acc[:, col : col + 1],
                                )
                            else:
                                nc.scalar.activation(
                                    out=dtile[:],
                                    in_=dtile[:],
                                    func=AF.Exp,
                                    scale=SCALE,
                                    accum_out=lse_acc[:, col : col + 1],
                                )

            # ---- tail: Gram-chain aux (deep-buffered) overlapped with the
            # lse -> Ln scalar work, then rank-1 + residual ------------------
            with tc.tile_pool(name="tail_ps", bufs=1, space="PSUM") as tps:
                def tail_tile():
                    return tps.tile([128, 512], f32, name="tsc", tag="tsc", bufs=8)

                # scalar/DVE lse work first: overlaps the PE aux below
                la = lse_acc[:].rearrange("q (p two) -> q p two", two=2)
                nc.vector.tensor_add(lse_sum[:], la[:, :, 0], la[:, :, 1])
                nc.scalar.activation(out=lse_ln[:], in_=lse_sum[:], func=AF.Ln)
                # lse_ln cols are h*8+t; gather per-t slices into t-major
                lse_tm = const.tile([128, 64], bf16, tag="lse_tm")
                nc.vector.tensor_copy(
                    lse_tm[:],
                    lse_ln[:].rearrange("q (h t) -> q t h", t=QT),
                )
                nu = 0
                while tail_rides:
                    tail_rides.pop(0)(tail_tile()[:])
                    nu += 1
                    if dve_rides and nu % 2 == 0:
                        dve_rides.pop(0)()
                while dve_rides:
                    dve_rides.pop(0)()
                for t in range(QT):
                    ps = tail_tile()
                    ps_bf = ps[0:8, 0:64].bitcast(bf16)
                    nc.tensor.transpose(ps_bf, lse_tm[:, t * 8 : (t + 1) * 8], ident_bf[:])
                    nc.vector.tensor_copy(lnST[:, t * 128 : (t + 1) * 128], ps_bf)
                    yps = tail_tile()
                    for c in range(4):
                        nc.tensor.matmul(
                            yps[:],
                            lhsT=qT[c][:, t * 128 : (t + 1) * 128],
                            rhs=KVW[c][:],
                            start=(c == 0),
                            stop=False,
                        )
                    nc.tensor.matmul(
                        yps[:],
                        lhsT=lnST[:, t * 128 : (t + 1) * 128],
                        rhs=W8_sb[:],
                        start=False,
                        stop=True,
                    )
                    ysb = dout.tile([128, D], f32, name="ysb", tag="ysb")
                    nc.vector.tensor_add(ysb[:], yps[:], xb[t][:])
                    nc.sync.dma_start(out=out_d[t * 128 : (t + 1) * 128, :], in_=ysb[:])

    nc.compile()
    return nc


def get_graph():
    if "nc" not in _GRAPH_CACHE:
        _GRAPH_CACHE["nc"] = _build_graph()
    return _GRAPH_CACHE["nc"]


def make_in_maps(x, w_qkv, w_out, b_out):
    import ml_dtypes

    x = np.ascontiguousarray(x, dtype=np.float32)
    w_qkv = np.ascontiguousarray(w_qkv, dtype=np.float32)
    w_out = np.ascontiguousarray(w_out, dtype=np.float32)
    b_out = np.ascontiguousarray(b_out, dtype=np.float32)
    x_bf = x.astype(ml_dtypes.bfloat16)
    w_qkv_bf = w_qkv.astype(ml_dtypes.bfloat16)
    w_out_bf = w_out.astype(ml_dtypes.bfloat16)
    in_maps = []
    for i in range(8):
        b, q = divmod(i, 4)
        in_maps.append(
            {
                # keys are permutation-invariant for lse/kv/G; roll so this
                # core's own query rows sit at rows 0:NQ
                "x_bf": np.ascontiguousarray(np.roll(x_bf[b], -q * NQ, axis=0)),
                "xq": np.ascontiguousarray(x[b, q * NQ : (q + 1) * NQ]),
                "w_qkv_bf": w_qkv_bf,
                "w_out_bf": w_out_bf,
                "b_out": b_out,
            }
        )
    return in_maps


def kernel(x, w_qkv, w_out, b_out):
    from concourse.bass_utils import run_bass_kernel_spmd

    nc = get_graph()
    in_maps = make_in_maps(x, w_qkv, w_out, b_out)
    res = run_bass_kernel_spmd(nc, in_maps, core_ids=list(range(8)))
    out = np.empty((B, N, D), np.float32)
    for i in range(8):
        b, q = divmod(i, 4)
        out[b, q * NQ : (q + 1) * NQ] = res.results[i]["out"]
    return out



# revision 10
# speedup vs baseline: 1.1193x; 1.1115x over previous
"""Trainium2 Bass kernel for log-softmax multi-head attention (8 NeuronCores).

Reference computation (per batch):
    qkv = x @ w_qkv ; q,k,v per head
    dots = scale * q @ k^T ; attn = log_softmax(dots)
    out = attn @ v  -> merge heads -> out @ w_out + b_out + x

Algebraic identities used:
  1) log_softmax is linear in scores minus a row constant:
       attn = scale*dots - lse,  lse_i = ln sum_j exp(scale*dots_ij)
     so  out_head = scale * q @ (k^T v) - lse (x) colsum(v)
  2) k^T v = Wk^T (x^T x) Wv  (Gram matrix G = x^T x shared by all heads)
  3) colsum(v) = colsum(x) @ Wv
  4) the lse rank-1 correction commutes with the output projection
  so the only O(n^2) work is the score matmul + exp/row-sum pass.

Sharding: 8 cores = 2 batches x 4 query-quarters, outputs disjoint.

Schedule (v2): the exp+rowsum pass is split across ScalarE (exact Exp
ACTIVATE, ~0.95ns/elem + 455ns/instr) and DVE (Schraudolph bit-trick:
i16 = trunc(raw*SC16+SB16) is bf16-bitcast ~exp(SCALE*raw); one 1x pass
from PSUM + a 2x bf16 tensor_tensor halving tree + small cache-reduce).
PSUM = 2 stream slots of [128,1536] (3 banks each) + 2 ride banks, so
score tiles are (1536,1536,1024) pieces per 128-row block.  kT/qT/Gram/
GWk/kv ride on the 2 spare banks during the stream (Gram accumulates
32-matmul chains fully in PSUM); rides keep PE continuously busy which
holds it at the 2.4 GHz p-state (idle PE decays to 1.2 GHz).  Tail does
csx/vsum/W8/Ln + rank-1 correction + output projection as before.
"""

import numpy as np

B, N, D = 2, 4096, 512
H, DH = 8, 64
SCALE = DH**-0.5
NQ = N // 4        # own query rows per core
QT = NQ // 128     # 8 own row tiles
NXT = N // 128     # 32 x row tiles

# Schraudolph fast-exp in bf16 (folding the 1/sqrt(DH) score scale):
#   i16 = trunc(raw * SC16 + SB16); bitcast_bf16(i16) ~ exp(SCALE*raw)
# c16=6.9 zeros the mean relative error (max ~4%/elem, ~0.2% on lse)
SC16 = float(np.float32(SCALE * (2.0**7) / np.log(2.0)))
SB16 = float(np.float32(127.0 * 2.0**7 - 6.9))

# stream piece layout per 128-query-row block: key ranges
PIECES = [(0, 1536), (1536, 3072), (3072, 4096)]

_GRAPH_CACHE = {}


def _build_graph():
    import concourse.bass as bass
    import concourse.tile as tile
    from concourse import bacc, mybir
    from concourse.masks import make_identity

    f32 = mybir.dt.float32
    bf16 = mybir.dt.bfloat16
    i16 = mybir.dt.int16
    AF = mybir.ActivationFunctionType
    ALU = mybir.AluOpType

    nc = bacc.Bacc("TRN2", target_bir_lowering=False, debug=False)

    xbf_d = nc.dram_tensor("x_bf", [N, D], bf16, kind="ExternalInput").ap()
    xq_d = nc.dram_tensor("xq", [NQ, D], f32, kind="ExternalInput").ap()
    wqkv_d = nc.dram_tensor("w_qkv_bf", [D, 3 * D], bf16, kind="ExternalInput").ap()
    wout_d = nc.dram_tensor("w_out_bf", [D, D], bf16, kind="ExternalInput").ap()
    bout_d = nc.dram_tensor("b_out", [D], f32, kind="ExternalInput").ap()
    out_d = nc.dram_tensor("out", [NQ, D], f32, kind="ExternalOutput").ap()

    with tile.TileContext(nc) as tc:
        with (
            tc.tile_pool(name="const", bufs=1) as const,
            tc.tile_pool(name="bigsb", bufs=1) as bigsb,
            tc.tile_pool(name="dout", bufs=2) as dout,
        ):
            # ------- constants + DMAs (issue order = priority) --------------
            ident_bf = const.tile([128, 128], bf16, tag="ident_bf")
            make_identity(nc, ident_bf[:])
            wq = []
            for j in range(4):
                w_t = const.tile([128, 3 * D], bf16, tag=f"wq{j}")
                nc.sync.dma_start(out=w_t[:], in_=wqkv_d[j * 128 : (j + 1) * 128, :])
                wq.append(w_t)
            # transposed x (kT/qT source): waves 0,1 first -> kT0 ch0-3/qT0
            xT = [bigsb.tile([128, N], bf16, name=f"xT{j}", tag=f"xT{j}") for j in range(4)]
            for r in range(2):
                for j in range(4):
                    nc.sync.dma_start(
                        out=xT[j][:, r * 1024 : (r + 1) * 1024],
                        in_=xbf_d[r * 1024 : (r + 1) * 1024, j * 128 : (j + 1) * 128],
                        transpose=True,
                    )
            # x row tiles t0-15 (Gram source, consumed by in-stream rides)
            xrow = []
            for t in range(16):
                xr_t = bigsb.tile([128, D], bf16, tag=f"xrow{t}")
                nc.sync.dma_start(out=xr_t[:], in_=xbf_d[t * 128 : (t + 1) * 128, :])
                xrow.append(xr_t)
            for r in range(2, 4):
                for j in range(4):
                    nc.sync.dma_start(
                        out=xT[j][:, r * 1024 : (r + 1) * 1024],
                        in_=xbf_d[r * 1024 : (r + 1) * 1024, j * 128 : (j + 1) * 128],
                        transpose=True,
                    )
            for t in range(16, NXT):
                xr_t = bigsb.tile([128, D], bf16, tag=f"xrow{t}")
                nc.sync.dma_start(out=xr_t[:], in_=xbf_d[t * 128 : (t + 1) * 128, :])
                xrow.append(xr_t)
            wo = []
            for j in range(4):
                w_t = const.tile([128, D], bf16, tag=f"wo{j}")
                nc.sync.dma_start(out=w_t[:], in_=wout_d[j * 128 : (j + 1) * 128, :])
                wo.append(w_t)
            b_bc = const.tile([128, D], f32, tag="b_bc")
            nc.sync.dma_start(
                out=b_bc[:],
                in_=bass.AP(
                    tensor=bout_d.tensor,
                    offset=bout_d.offset,
                    ap=[[0, 128]] + [list(p) for p in bout_d.ap],
                ),
            )
            # residual rows (f32) -> become x + b via DVE adds in tail
            xb = []
            for t in range(QT):
                xb_t = dout.tile([128, D], f32, tag=f"xb{t}", bufs=1)
                nc.sync.dma_start(out=xb_t[:], in_=xq_d[t * 128 : (t + 1) * 128, :])
                xb.append(xb_t)

            # ------- big SBUF operands --------------------------------------
            qT = [bigsb.tile([128, NQ], bf16, name=f"qT{c}", tag=f"qT{c}") for c in range(4)]
            kT = [bigsb.tile([128, N], bf16, name=f"kT{c}", tag=f"kT{c}") for c in range(4)]
            G_bf = [bigsb.tile([128, D], bf16, name=f"Gb{j}", tag=f"Gb{j}") for j in range(4)]
            GWk = [bigsb.tile([128, D], bf16, name=f"GWk{j}", tag=f"GWk{j}") for j in range(4)]
            KVW = [bigsb.tile([128, D], bf16, name=f"KVW{c}", tag=f"KVW{c}") for c in range(4)]
            kv_p = const.tile([128, D], bf16, tag="kv_p")
            nc.vector.memset(kv_p[:], 0.0)
            csx4 = [const.tile([128, 4], f32, name=f"csx4_{j}", tag=f"csx4_{j}") for j in range(4)]
            csx_bf = [const.tile([128, 1], bf16, name=f"csxb{j}", tag=f"csxb{j}") for j in range(4)]
            vsT = [const.tile([128, 1], bf16, name=f"vsT{j}", tag=f"vsT{j}") for j in range(4)]
            VSmat = [const.tile([128, 8], bf16, name=f"VSm{j}", tag=f"VSm{j}") for j in range(4)]
            for j in range(4):
                nc.vector.memset(VSmat[j][:], 0.0)
            W8_sb = const.tile([8, D], bf16, tag="W8")
            # lse accumulator: col = (h*8+t)*3 + piece
            lse_acc = const.tile([128, 192], f32, tag="lse_acc")
            lse_sum = const.tile([128, 64], f32, tag="lse_sum")
            lse_ln = const.tile([128, 64], bf16, tag="lse_ln")
            lnST = const.tile([8, NQ], bf16, tag="lnST")
            dummy = const.tile([128, 1], f32, tag="dummy")
            nc.vector.memset(dummy[:], 0.0)
            # bit-trick scratch
            scr_i16 = const.tile([128, 1536], i16, tag="scr_i16")
            tr1 = const.tile([128, 768], bf16, tag="tr1")
            tr2 = const.tile([128, 384], bf16, tag="tr2")
            tr3 = const.tile([128, 384], bf16, tag="tr3")

            # preload the Exp table set before the stream
            nc.scalar.activation(out=dummy[:], in_=dummy[:], func=AF.Exp)

            # =================================================================
            # The single PSUM layout for the whole kernel:
            #   P[:, 0:1536]     slot 0 (3 banks)
            #   P[:, 1536:3072]  slot 1 (3 banks)
            #   P[:, 3072:3584]  ride bank A
            #   P[:, 3584:4096]  ride bank B
            # =================================================================
            with tc.tile_pool(name="allps", bufs=1, space="PSUM") as aps:
                P = aps.tile([128, 4096], f32, tag="P")
                slot = [P[:, 0:1536], P[:, 1536:3072]]
                rbank = [P[:, 3072:3584], P[:, 3584:4096]]
                rb_bf = [r.bitcast(bf16) for r in rbank]

                # ---- ride bodies (write into ride bank rb in {0,1}) --------
                def kT_chunk(rb, c, ch):
                    for j in range(4):
                        nc.tensor.matmul(
                            rbank[rb],
                            lhsT=wq[j][:, 512 + c * 128 : 512 + (c + 1) * 128],
                            rhs=xT[j][:, ch * 512 : (ch + 1) * 512],
                            start=(j == 0),
                            stop=(j == 3),
                        )
                    nc.vector.tensor_copy(kT[c][:, ch * 512 : (ch + 1) * 512], rbank[rb])

                def kT_chunk_act(rb, c, ch):
                    for j in range(4):
                        nc.tensor.matmul(
                            rbank[rb],
                            lhsT=wq[j][:, 512 + c * 128 : 512 + (c + 1) * 128],
                            rhs=xT[j][:, ch * 512 : (ch + 1) * 512],
                            start=(j == 0),
                            stop=(j == 3),
                        )
                    nc.scalar.activation(
                        out=kT[c][:, ch * 512 : (ch + 1) * 512], in_=rbank[rb],
                        func=AF.Copy,
                    )

                def qT_half(rb, c, nn, drain):
                    for j in range(4):
                        nc.tensor.matmul(
                            rbank[rb],
                            lhsT=wq[j][:, c * 128 : (c + 1) * 128],
                            rhs=xT[j][:, nn * 512 : (nn + 1) * 512],
                            start=(j == 0),
                            stop=(j == 3),
                        )
                    if drain == 0:
                        nc.vector.tensor_copy(qT[c][:, nn * 512 : (nn + 1) * 512], rbank[rb])
                    else:
                        nc.scalar.activation(
                            out=qT[c][:, nn * 512 : (nn + 1) * 512], in_=rbank[rb],
                            func=AF.Copy,
                        )

                def gram_chain(rb, jm, drain):
                    # full 32-matmul accumulation chain in the ride bank
                    for t in range(NXT):
                        nc.tensor.matmul(
                            rbank[rb],
                            lhsT=xrow[t][:, jm * 128 : (jm + 1) * 128],
                            rhs=xrow[t][:],
                            start=(t == 0),
                            stop=(t == NXT - 1),
                        )
                    if drain == 0:
                        nc.vector.tensor_copy(G_bf[jm][:], rbank[rb])
                    else:
                        nc.scalar.activation(out=G_bf[jm][:], in_=rbank[rb], func=AF.Copy)

                def gwk_jm(rb, jm, drain):
                    for j in range(4):
                        nc.tensor.matmul(
                            rbank[rb],
                            lhsT=G_bf[j][:, jm * 128 : (jm + 1) * 128],
                            rhs=wq[j][:, 512:1024],
                            start=(j == 0),
                            stop=(j == 3),
                        )
                    if drain == 0:
                        nc.vector.tensor_copy(GWk[jm][:], rbank[rb])
                    else:
                        nc.scalar.activation(out=GWk[jm][:], in_=rbank[rb], func=AF.Copy)

                def kvt_head(rb, h):
                    # kv^T_h = Wv_h^T (G Wk)_h ; scaled into kv_p rows
                    r0 = (h % 2) * 64
                    for j in range(4):
                        nc.tensor.matmul(
                            rbank[rb][0:64, 0:64],
                            lhsT=wq[j][:, 1024 + h * 64 : 1024 + (h + 1) * 64],
                            rhs=GWk[j][:, h * 64 : (h + 1) * 64],
                            start=(j == 0),
                            stop=(j == 3),
                        )
                    nc.vector.tensor_scalar_mul(
                        kv_p[r0 : r0 + 64, h * 64 : (h + 1) * 64],
                        rbank[rb][0:64, 0:64], SCALE,
                    )

                # ---- head: kT0 ch0-3 + qT0 (xT waves 0-1 + wq) -------------
                for ch in range(4):
                    (kT_chunk if ch % 2 == 0 else kT_chunk_act)(ch % 2, 0, ch)
                qT_half(0, 0, 0, 0)
                qT_half(1, 0, 1, 1)

                # ---- ride schedule: (slot-free) rides through the 2 banks --
                rides = []
                for ch in range(4, 8):                       # kT0 tail
                    rides.append(lambda rb, ch=ch, d=ch % 2: (kT_chunk if d == 0 else kT_chunk_act)(rb, 0, ch))
                for c in range(1, 4):                        # kT1-3 + qT1-3
                    for ch in range(8):
                        rides.append(lambda rb, c=c, ch=ch, d=(ch + c) % 2: (kT_chunk if d == 0 else kT_chunk_act)(rb, c, ch))
                    rides.append(lambda rb, c=c: qT_half(rb, c, 0, 0))
                    rides.append(lambda rb, c=c: qT_half(rb, c, 1, 1))
                for jm in range(4):                          # Gram
                    rides.append(lambda rb, jm=jm, d=jm % 2: gram_chain(rb, jm, d))
                for jm in range(4):                          # G @ Wk
                    rides.append(lambda rb, jm=jm, d=jm % 2: gwk_jm(rb, jm, d))
                for h in range(H):                           # kv per head
                    rides.append(lambda rb, h=h: kvt_head(rb, h))

                # ---- the exp stream: (h, piece, t); rides paced in --------
                # ACT/DVE assignment: DVE takes DVE_SH of 192 pieces
                DVE_SH = 72
                nxt_ride = [0]

                def pace_rides(k):
                    for _ in range(k):
                        if nxt_ride[0] < len(rides):
                            rides[nxt_ride[0]](nxt_ride[0] % 2)
                            nxt_ride[0] += 1

                pidx = 0
                for h in range(H):
                    c, r0 = h // 2, (h % 2) * 64
                    for piece, (k0, k1) in enumerate(PIECES):
                        for t in range(QT):
                            s = slot[pidx % 2]
                            fd = k1 - k0
                            lhsT = qT[c][r0 : r0 + 64, t * 128 : (t + 1) * 128]
                            for cc in range((fd + 511) // 512):
                                nc.tensor.matmul(
                                    s[:, cc * 512 : min((cc + 1) * 512, fd)],
                                    lhsT=lhsT,
                                    rhs=kT[c][r0 : r0 + 64, k0 + cc * 512 : min(k0 + (cc + 1) * 512, k1)],
                                    start=True,
                                    stop=True,
                                )
                            col = (h * 8 + t) * 3 + piece
                            if (pidx * DVE_SH) // 192 != ((pidx + 1) * DVE_SH) // 192:
                                # DVE bit-trick path
                                nc.vector.tensor_scalar(
                                    out=scr_i16[:, 0:fd],
                                    in0=s[:, 0:fd],
                                    scalar1=SC16,
                                    scalar2=SB16,
                                    op0=ALU.mult,
                                    op1=ALU.add,
                                )
                                hf = fd // 2
                                nc.vector.tensor_tensor(
                                    out=tr1[:, 0 : hf // 1],
                                    in0=scr_i16[:, 0:hf].bitcast(bf16),
                                    in1=scr_i16[:, hf:fd].bitcast(bf16),
                                    op=ALU.add,
                                )
                                qf = hf // 2
                                nc.vector.tensor_tensor(
                                    out=tr2[:, 0:qf],
                                    in0=tr1[:, 0:qf],
                                    in1=tr1[:, qf:hf],
                                    op=ALU.add,
                                )
                                nc.vector.tensor_scalar(
                                    out=tr3[:, 0:qf],
                                    in0=tr2[:, 0:qf],
                                    scalar1=1.0,
                                    scalar2=None,
                                    op0=ALU.mult,
                                    op1=ALU.add,
                                    accum_out=lse_acc[:, col : col + 1],
                                )
                            else:
                                nc.scalar.activation(
                                    out=s[:, 0:fd],
                                    in_=s[:, 0:fd],
                                    func=AF.Exp,
                                    scale=SCALE,
                                    accum_out=lse_acc[:, col : col + 1],
                                )
                            pidx += 1
                            if pidx % 3 == 0:
                                pace_rides(1)
                while nxt_ride[0] < len(rides):
                    pace_rides(1)

                # ---- tail: csx/vsum/W8 + Ln + rank-1 + projection ----------
                # lse: sum the 3 per-piece accumulator cols -> [128, 64]
                la = lse_acc[:].rearrange("q (p three) -> q p three", three=3)
                nc.vector.tensor_add(lse_sum[:], la[:, :, 0], la[:, :, 1])
                nc.vector.tensor_add(lse_sum[:], lse_sum[:], la[:, :, 2])
                nc.scalar.activation(out=lse_ln[:], in_=lse_sum[:], func=AF.Ln)
                lse_tm = const.tile([128, 64], bf16, tag="lse_tm")
                nc.vector.tensor_copy(
                    lse_tm[:],
                    lse_ln[:].rearrange("q (h t) -> q t h", t=QT),
                )
                # csx pieces (DVE) + xb adds
                for j in range(4):
                    for p in range(4):
                        nc.vector.tensor_reduce(
                            csx4[j][:, p : p + 1],
                            xT[j][:, p * 1024 : (p + 1) * 1024],
                            axis=mybir.AxisListType.X,
                            op=ALU.add,
                        )
                for j in range(4):
                    nc.vector.tensor_reduce(
                        csx4[j][:, 0:1], csx4[j][:],
                        axis=mybir.AxisListType.X, op=ALU.add,
                    )
                    nc.vector.tensor_copy(csx_bf[j][:], csx4[j][:, 0:1])
                for t in range(QT):
                    nc.vector.tensor_add(xb[t][:], xb[t][:], b_bc[:])

                def vsum_jm(rb, jm):
                    for j in range(4):
                        nc.tensor.matmul(
                            rbank[rb][:, 0:1],
                            lhsT=wq[j][:, 1024 + jm * 128 : 1024 + (jm + 1) * 128],
                            rhs=csx_bf[j][:],
                            start=(j == 0),
                            stop=(j == 3),
                        )
                    nc.vector.tensor_scalar_mul(vsT[jm][:], rbank[rb][:, 0:1], -1.0)
                    nc.vector.tensor_copy(
                        VSmat[jm][0:64, 2 * jm : 2 * jm + 1], vsT[jm][0:64, :]
                    )
                    nc.vector.tensor_copy(
                        VSmat[jm][64:128, 2 * jm + 1 : 2 * jm + 2], vsT[jm][64:128, :]
                    )

                for jm in range(4):
                    vsum_jm(jm % 2, jm)
                for j in range(4):
                    nc.tensor.matmul(
                        rbank[0][0:8, :],
                        lhsT=VSmat[j][:],
                        rhs=wo[j][:],
                        start=(j == 0),
                        stop=(j == 3),
                    )
                nc.vector.tensor_copy(W8_sb[:], rbank[0][0:8, :])
                for c in range(4):
                    # KVW_c = scale * kv_h @ Wo_h for both heads of c
                    for hp in range(2):
                        h, r0 = 2 * c + hp, hp * 64
                        nc.tensor.matmul(
                            rbank[1][r0 : r0 + 64, :],
                            lhsT=kv_p[:, h * 64 : (h + 1) * 64],
                            rhs=wo[c][:],
                            start=True,
                            stop=True,
                        )
                    nc.vector.tensor_copy(KVW[c][:], rbank[1])

                for t in range(QT):
                    sl = slot[t % 2]
                    ps_bf = sl[0:8, 0:64].bitcast(bf16)
                    nc.tensor.transpose(ps_bf, lse_tm[:, t * 8 : (t + 1) * 8], ident_bf[:])
                    nc.vector.tensor_copy(lnST[:, t * 128 : (t + 1) * 128], ps_bf)
                    yps = sl[:, 512:1024]
                    for c in range(4):
                        nc.tensor.matmul(
                            yps,
                            lhsT=qT[c][:, t * 128 : (t + 1) * 128],
                            rhs=KVW[c][:],
                            start=(c == 0),
                            stop=False,
                        )
                    nc.tensor.matmul(
                        yps,
                        lhsT=lnST[:, t * 128 : (t + 1) * 128],
                        rhs=W8_sb[:],
                        start=False,
                        stop=True,
                    )
                    ysb = dout.tile([128, D], f32, name="ysb", tag="ysb")
                    nc.vector.tensor_add(ysb[:], yps, xb[t][:])
                    nc.sync.dma_start(out=out_d[t * 128 : (t + 1) * 128, :], in_=ysb[:])

    nc.compile()
    return nc


def get_graph():
    if "nc" not in _GRAPH_CACHE:
        _GRAPH_CACHE["nc"] = _build_graph()
    return _GRAPH_CACHE["nc"]


def make_in_maps(x, w_qkv, w_out, b_out):
    import ml_dtypes

    x = np.ascontiguousarray(x, dtype=np.float32)
    w_qkv = np.ascontiguousarray(w_qkv, dtype=np.float32)
    w_out = np.ascontiguousarray(w_out, dtype=np.float32)
    b_out = np.ascontiguousarray(b_out, dtype=np.float32)
    x_bf = x.astype(ml_dtypes.bfloat16)
    w_qkv_bf = w_qkv.astype(ml_dtypes.bfloat16)
    w_out_bf = w_out.astype(ml_dtypes.bfloat16)
    in_maps = []
    for i in range(8):
        b, q = divmod(i, 4)
        in_maps.append(
            {
                # keys are permutation-invariant for lse/kv/G; roll so this
                # core's own query rows sit at rows 0:NQ
                "x_bf": np.ascontiguousarray(np.roll(x_bf[b], -q * NQ, axis=0)),
                "xq": np.ascontiguousarray(x[b, q * NQ : (q + 1) * NQ]),
                "w_qkv_bf": w_qkv_bf,
                "w_out_bf": w_out_bf,
                "b_out": b_out,
            }
        )
    return in_maps


def kernel(x, w_qkv, w_out, b_out):
    from concourse.bass_utils import run_bass_kernel_spmd

    nc = get_graph()
    in_maps = make_in_maps(x, w_qkv, w_out, b_out)
    res = run_bass_kernel_spmd(nc, in_maps, core_ids=list(range(8)))
    out = np.empty((B, N, D), np.float32)
    for i in range(8):
        b, q = divmod(i, 4)
        out[b, q * NQ : (q + 1) * NQ] = res.results[i]["out"]
    return out
